# revision 29
# baseline (speedup 1.0000x reference)
"""CTC loss (keras ctc_batch_cost semantics) on 8 Trainium2 NeuronCores.

Data-parallel over batch: 1024 samples -> 8 cores x 128 samples
(one sample per SBUF partition).  Host prep is integer-only (gather
index tables + skip masks); all float work runs on device.

Device pipeline (per core):
  A. per 4-sample group x 2 T-halves: DMA load [t,(4b,c)] f32, DVE
     cast fp32->bf16 (+EPS), 4x PE transpose -> PSUM [c,t] f32, ACT
     copy PSUM->SBUF bf16; one contiguous 256KB store per group to an
     interleaved HBM scratch (row(b,c) = (b//4)*512 + c*4 + b%4, 512B
     full-T rows -> large DMA runs both directions).
  B. 9 SWDGE full-T gathers (elem 512B): rows (b, blank) and
     (b, lab_j) -> ptil[b, slot*T..] (slot 0 = blank, 1+j = label j).
  C. per-column scale from gather 0: gmax = max over slots 0..7,
     ginv = e^RHAT/gmax (fp32 recip, bf16); ptil = raw * ginv.
  D. s-sweep over the 129-row CTC lattice in 2 T-chunks, wavefront
     pipelined: row coupling (odd rows) as scalar_tensor_tensor on
     GpSimd, recursion v_t = (e_t + v_{t-1}) * p_t as
     tensor_tensor_scan on DVE (fp32 carry, bf16 treg).  Head/tail
     pruned to the reachable lattice region.  No mid-scan renorm:
     RHAT=0.28 centers the running product inside fp32 range.
Host assembles loss = -(log raw - sum log ginv) in f64.
"""
from contextlib import ExitStack

import numpy as np

import concourse.bass as bass
import concourse.tile as tile
from concourse import bacc, mybir, masks
from concourse.bass_utils import run_bass_kernel_spmd

F32 = mybir.dt.float32
BF16 = mybir.dt.bfloat16
I16 = mybir.dt.int16
AF = mybir.ActivationFunctionType
ALU = mybir.AluOpType

B, T, C, L = 1024, 256, 128, 64
S = 2 * L + 1           # 129 extended states
NBLK = L + 1            # blank slot + 64 label slots
BLANK = C - 1
EPS = 1e-7
RHAT = 0.24             # per-step prob boost; keeps alpha in fp32 range
TC = 128                # T-chunk for the scan wavefront
W = T + 1               # treg slot width: col0 = v_{-1}, col 1+t = v_t
SLOTS = S + 2           # 2 permanent zero rows + 129 state rows
PB = 128                # samples per core
NCORES = 8
SG = 4                  # samples per load/transpose group
NG = PB // SG           # 32 groups


def _scratch_row(b, ch):
    return (b // SG) * (C * SG) + ch * SG + b % SG


def _host_prep(y_true_shard: np.ndarray):
    yt = y_true_shard.astype(np.int64)
    barange = np.arange(PB)
    # per-partition row indices for the indirect gather: bidx[b, j] = row of
    # (sample b, label j) in the interleaved scratch
    bidx = np.empty((PB, L), np.int32)
    for j in range(L):
        bidx[:, j] = _scratch_row(barange, yt[:, j])
    m01 = np.ones((PB, L), np.float32)
    m01[:, 1:] = (yt[:, 1:] != yt[:, :-1]).astype(np.float32)
    m01[:, 0] = 0.0
    return {"bidx": bidx, "m01": m01}


def _emit(ctx: ExitStack, tc: tile.TileContext, y_in, bidx_in, m01_in,
          raw_out, ginv_out):
    nc = tc.nc

    persist = ctx.enter_context(tc.tile_pool(name="persist", bufs=1))
    stage = ctx.enter_context(tc.tile_pool(name="stage", bufs=6))
    stg2 = ctx.enter_context(tc.tile_pool(name="stg2", bufs=4))
    praw = ctx.enter_context(tc.tile_pool(name="praw", bufs=4))
    cpool = ctx.enter_context(tc.tile_pool(name="cbuf", bufs=4))
    psum = ctx.enter_context(
        tc.tile_pool(name="psum", bufs=8, space=bass.MemorySpace.PSUM))
    dram = ctx.enter_context(tc.tile_pool(name="dram", bufs=1, space="DRAM"))

    bidx = persist.tile([PB, L], mybir.dt.int32)
    nc.sync.dma_start(bidx[:], bidx_in[:])
    m01 = persist.tile([PB, L], F32)
    nc.sync.dma_start(m01[:], m01_in[:])

    ident = persist.tile([128, 128], F32)
    masks.make_identity(nc, ident[:])
    epsb = persist.tile([PB, 1], F32)
    nc.vector.memset(epsb[:], EPS)

    treg = persist.tile([PB, SLOTS * W], BF16)
    # only the read-before-write region needs zeroing: the two permanent
    # zero rows, plus cols [0, 65) of every slot (covers col0 + the pruned
    # head triangle; max unwritten-but-read col is t0(s) <= 64)
    nc.vector.memset(treg[:, 0:2 * W], 0.0)
    nc.vector.memset(treg[:].rearrange("p (s w) -> p s w", w=W)[:, :, 0:65],
                     0.0)

    ptil = persist.tile([PB, NBLK * T], BF16)
    gmax = persist.tile([PB, T], F32)
    ginvb = persist.tile([PB, T], BF16)
    raw = persist.tile([PB, 1], F32)

    scratch = dram.tile([PB * C, T], BF16)

    # ---- Phase A: load -> PE transpose -> copy(+EPS, ->bf16) -> store
    for g in range(NG):
        b0 = g * SG
        st2 = stg2.tile([PB, SG, T], BF16, tag="st2")
        for h in range(2):
            ld = stage.tile([PB, SG * C], F32, tag="ld")
            eng = nc.sync if (2 * g + h) % 2 == 0 else nc.scalar
            eng.dma_start(
                ld[:].rearrange("p (b c) -> p b c", b=SG),
                y_in[b0:b0 + SG, h * TC:(h + 1) * TC, :]
                .rearrange("b t c -> t b c"))
            pt = psum.tile([PB, SG, TC], F32, tag="pt")
            for i in range(SG):
                nc.tensor.transpose(pt[:, i, :], ld[:, i * C:(i + 1) * C],
                                    ident[:])
            # PSUM -> SBUF (+EPS, downcast), alternating DVE / ACT
            if h == 0:
                nc.vector.tensor_scalar_add(st2[:, :, h * TC:(h + 1) * TC],
                                            pt[:], EPS)
            else:
                nc.scalar.activation(st2[:, :, h * TC:(h + 1) * TC], pt[:],
                                     AF.Identity, bias=epsb[:, 0:1])
        eng = nc.sync if g % 2 == 0 else nc.scalar
        eng.dma_start(
            scratch[g * (C * SG):(g + 1) * (C * SG), :]
            .rearrange("(c bl) t -> c bl t", bl=SG),
            st2[:])

    # ---- Phase B/C interleaved with phase D (wavefront over gather batches)
    # blank rows (b, BLANK) are a fixed strided pattern -> plain DMA
    prb = praw.tile([PB, 8, T], BF16, tag="prb")
    nc.sync.dma_start(
        prb[:, 0, :],
        scratch[:].rearrange("(g cb) t -> g cb t", cb=C * SG)
        [:, BLANK * SG:BLANK * SG + SG, :])
    # batch q gathers label slots 8q..8q+7 -> praw[:, i, :]
    # (indirect DMA supports one offset per partition per call).  Batches
    # are issued lazily, interleaved with phase D on the Pool queue.
    prq: dict = {}

    def gather_batch(q):
        pr = praw.tile([PB, 8, T], BF16, tag="pr")
        for i in range(8):
            nc.gpsimd.indirect_dma_start(
                out=pr[:, i, :],
                out_offset=None,
                in_=scratch[:],
                in_offset=bass.IndirectOffsetOnAxis(
                    ap=bidx[:, 8 * q + i:8 * q + i + 1], axis=0))
        prq[q] = pr

    gather_batch(0)
    gather_batch(1)

    # gmax over blank + labels j0..j7 (from blank DMA + batch 0)
    nc.vector.tensor_reduce(
        gmax[:], prq[0][:].rearrange("p blk t -> p t blk"),
        axis=mybir.AxisListType.X, op=ALU.max)
    nc.vector.tensor_tensor(gmax[:], gmax[:], prb[:, 0, :], op=ALU.max)
    nc.vector.tensor_scalar_mul(gmax[:], gmax[:], float(np.exp(-RHAT)))
    ginv32 = cpool.tile([PB, T], F32, tag="ginv32")
    nc.vector.reciprocal(ginv32[:], gmax[:])
    nc.vector.tensor_copy(ginvb[:], ginv32[:])
    nc.sync.dma_start(ginv_out[:], ginvb[:])
    # blank slot scale
    nc.vector.tensor_mul(ptil[:, 0:T], prb[:, 0, :], ginvb[:])

    # ---- Phase D: 129-row s-sweep, 2 chunks, wavefront; scale-muls for
    # gather batch q are emitted just before the rows that need them.
    def sb(s):
        return (s + 2) * W

    def scale_batch(q):
        s0 = (1 + q * 8) * T
        nc.vector.tensor_mul(
            ptil[:, s0:s0 + 8 * T].rearrange("p (a b) -> p a b", a=8),
            prq[q][:],
            ginvb[:].rearrange("p (o t) -> p o t", o=1).to_broadcast(
                [PB, 8, T]))

    for s in range(S):
        if s % 16 == 0 and s // 16 < 8:
            scale_batch(s // 16)
            if s // 16 + 2 < 8:
                gather_batch(s // 16 + 2)
        slot = 0 if s % 2 == 0 else 1 + (s - 1) // 2
        j = (s - 1) // 2
        t0 = 0 if s <= 1 else s // 2               # head prune
        t1 = T - (128 - s) // 2 if s < 128 else T  # tail prune
        n = t1 - t0
        if s % 2 == 1:
            # coupling c = m01_j * v^{s-2} + v^{s-1}: mask-mult on ACT
            # (dep on row s-2 -> off the serial chain), add on DVE
            c0 = cpool.tile([PB, T], BF16, tag="c")
            nc.scalar.activation(
                c0[:, :n],
                treg[:, sb(s - 2) + t0: sb(s - 2) + t1],
                AF.Identity, scale=m01[:, j:j + 1])
            nc.vector.tensor_add(
                c0[:, :n], c0[:, :n],
                treg[:, sb(s - 1) + t0: sb(s - 1) + t1])
            d0 = c0[:, :n]
        else:
            d0 = treg[:, sb(s - 1) + t0: sb(s - 1) + t1]
        nc.vector.tensor_tensor_scan(
            treg[:, sb(s) + 1 + t0: sb(s) + 1 + t1],
            d0, ptil[:, slot * T + t0: slot * T + t1],
            1.0 if s <= 1 else 0.0,
            op0=ALU.add, op1=ALU.mult)

    b127 = sb(127) + T
    b128 = sb(128) + T
    nc.vector.tensor_add(raw[:, 0:1], treg[:, b127:b127 + 1],
                         treg[:, b128:b128 + 1])
    nc.sync.dma_start(raw_out[:], raw[:])


_CACHE: dict = {}


def _build():
    nc = bacc.Bacc("TRN2", target_bir_lowering=False, debug=False,
                   num_devices=NCORES, num_swdge_queues=4)
    y_in = nc.dram_tensor("ypred", [PB, T, C], F32, kind="ExternalInput").ap()
    bidx_in = nc.dram_tensor("bidx", [PB, L], mybir.dt.int32,
                             kind="ExternalInput").ap()
    m01_in = nc.dram_tensor("m01", [PB, L], F32, kind="ExternalInput").ap()
    raw_out = nc.dram_tensor("raw", [PB, 1], F32, kind="ExternalOutput").ap()
    ginv_out = nc.dram_tensor("ginv", [PB, T], BF16, kind="ExternalOutput").ap()
    with tile.TileContext(nc) as tcx:
        with ExitStack() as ctx:
            _emit(ctx, tcx, y_in, bidx_in, m01_in, raw_out, ginv_out)
    nc.compile()
    return nc


def _run(in_maps, **kwargs):
    if "nc" not in _CACHE:
        _CACHE["nc"] = _build()
    return run_bass_kernel_spmd(_CACHE["nc"], in_maps,
                                core_ids=list(range(NCORES)), **kwargs)


def kernel(y_true: np.ndarray, y_pred: np.ndarray, **run_kwargs) -> np.ndarray:
    assert y_pred.shape == (B, T, C), y_pred.shape
    in_maps = []
    for c in range(NCORES):
        sl = slice(c * PB, (c + 1) * PB)
        prep = _host_prep(y_true[sl])
        in_maps.append({"ypred": np.ascontiguousarray(y_pred[sl], np.float32),
                        "bidx": prep["bidx"], "m01": prep["m01"]})
    res = _run(in_maps, **run_kwargs)
    raw = np.concatenate([res.results[c]["raw"] for c in range(NCORES)], axis=0)
    ginv = np.concatenate([res.results[c]["ginv"] for c in range(NCORES)],
                          axis=0).astype(np.float64)
    lng = np.log(ginv).sum(axis=1)
    loss = -(np.log(raw[:, 0].astype(np.float64)) - lng)
    if run_kwargs:
        kernel.last_results = res  # expose trace info to test harness
    return loss[:, None].astype(np.float32)


# revision 30
# speedup vs baseline: 1.0307x; 1.0307x over previous
"""CTC loss (keras ctc_batch_cost semantics) on 8 Trainium2 NeuronCores.

Data-parallel over batch: 1024 samples -> 8 cores x 128 samples
(one sample per SBUF partition).  Host prep is integer-only (gather
index tables + skip masks); all float work runs on device.

Device pipeline (per core):
  A. per 4-sample group x 2 T-halves: DMA load [t,(4b,c)] f32, DVE
     cast fp32->bf16 (+EPS), 4x PE transpose -> PSUM [c,t] f32, ACT
     copy PSUM->SBUF bf16; one contiguous 256KB store per group to an
     interleaved HBM scratch (row(b,c) = (b//4)*512 + c*4 + b%4, 512B
     full-T rows -> large DMA runs both directions).
  B. 9 SWDGE full-T gathers (elem 512B): rows (b, blank) and
     (b, lab_j) -> ptil[b, slot*T..] (slot 0 = blank, 1+j = label j).
  C. per-column scale from gather 0: gmax = max over slots 0..7,
     ginv = e^RHAT/gmax (fp32 recip, bf16); ptil = raw * ginv.
  D. s-sweep over the 129-row CTC lattice in 2 T-chunks, wavefront
     pipelined: row coupling (odd rows) as scalar_tensor_tensor on
     GpSimd, recursion v_t = (e_t + v_{t-1}) * p_t as
     tensor_tensor_scan on DVE (fp32 carry, bf16 treg).  Head/tail
     pruned to the reachable lattice region.  No mid-scan renorm:
     RHAT=0.28 centers the running product inside fp32 range.
Host assembles loss = -(log raw - sum log ginv) in f64.
"""
from contextlib import ExitStack

import numpy as np

import concourse.bass as bass
import concourse.tile as tile
from concourse import bacc, mybir, masks
from concourse.bass_utils import run_bass_kernel_spmd

F32 = mybir.dt.float32
BF16 = mybir.dt.bfloat16
I16 = mybir.dt.int16
AF = mybir.ActivationFunctionType
ALU = mybir.AluOpType

B, T, C, L = 1024, 256, 128, 64
S = 2 * L + 1           # 129 extended states
NBLK = L + 1            # blank slot + 64 label slots
BLANK = C - 1
EPS = 1e-7
RHAT = 0.24             # per-step prob boost; keeps alpha in fp32 range
TC = 128                # T-chunk for the scan wavefront
W = T + 1               # treg slot width: col0 = v_{-1}, col 1+t = v_t
SLOTS = S + 2           # 2 permanent zero rows + 129 state rows
PB = 128                # samples per core
NCORES = 8
SG = 4                  # samples per load/transpose group
NG = PB // SG           # 32 groups


def _scratch_row(b, ch):
    return (b // SG) * (C * SG) + ch * SG + b % SG


def _host_prep(y_true_shard: np.ndarray):
    yt = y_true_shard.astype(np.int64)
    barange = np.arange(PB)
    # per-partition row indices for the indirect gather: bidx[b, j] = row of
    # (sample b, label j) in the interleaved scratch
    bidx = np.empty((PB, L), np.int32)
    for j in range(L):
        bidx[:, j] = _scratch_row(barange, yt[:, j])
    m01 = np.ones((PB, L), np.float32)
    m01[:, 1:] = (yt[:, 1:] != yt[:, :-1]).astype(np.float32)
    m01[:, 0] = 0.0
    return {"bidx": bidx, "m01": m01}


def _emit(ctx: ExitStack, tc: tile.TileContext, y_in, bidx_in, m01_in,
          raw_out, ginv_out):
    nc = tc.nc

    persist = ctx.enter_context(tc.tile_pool(name="persist", bufs=1))
    stage = ctx.enter_context(tc.tile_pool(name="stage", bufs=6))
    stg2 = ctx.enter_context(tc.tile_pool(name="stg2", bufs=4))
    praw = ctx.enter_context(tc.tile_pool(name="praw", bufs=4))
    cpool = ctx.enter_context(tc.tile_pool(name="cbuf", bufs=4))
    psum = ctx.enter_context(
        tc.tile_pool(name="psum", bufs=8, space=bass.MemorySpace.PSUM))
    dram = ctx.enter_context(tc.tile_pool(name="dram", bufs=1, space="DRAM"))

    bidx = persist.tile([PB, L], mybir.dt.int32)
    nc.sync.dma_start(bidx[:], bidx_in[:])
    m01 = persist.tile([PB, L], F32)
    nc.sync.dma_start(m01[:], m01_in[:])

    ident = persist.tile([128, 128], F32)
    masks.make_identity(nc, ident[:])
    epsb = persist.tile([PB, 1], F32)
    nc.vector.memset(epsb[:], EPS)

    treg = persist.tile([PB, SLOTS * W], BF16)
    # only the read-before-write region needs zeroing: the two permanent
    # zero rows, plus cols [0, 65) of every slot (covers col0 + the pruned
    # head triangle; max unwritten-but-read col is t0(s) <= 64)
    nc.vector.memset(treg[:, 0:2 * W], 0.0)
    nc.vector.memset(treg[:].rearrange("p (s w) -> p s w", w=W)[:, :, 0:65],
                     0.0)

    ptil = persist.tile([PB, NBLK * T], BF16)
    gmax = persist.tile([PB, T], F32)
    ginvb = persist.tile([PB, T], BF16)
    raw = persist.tile([PB, 1], F32)

    scratch = dram.tile([PB * C, T], BF16)

    # ---- Phase A: load -> PE transpose -> copy(+EPS, ->bf16) -> store
    for g in range(NG):
        b0 = g * SG
        st2 = stg2.tile([PB, SG, T], BF16, tag="st2")
        for h in range(2):
            ld = stage.tile([PB, SG * C], F32, tag="ld")
            eng = nc.sync if (2 * g + h) % 2 == 0 else nc.scalar
            eng.dma_start(
                ld[:].rearrange("p (b c) -> p b c", b=SG),
                y_in[b0:b0 + SG, h * TC:(h + 1) * TC, :]
                .rearrange("b t c -> t b c"))
            pt = psum.tile([PB, SG, TC], F32, tag="pt")
            for i in range(SG):
                nc.tensor.transpose(pt[:, i, :], ld[:, i * C:(i + 1) * C],
                                    ident[:])
            # PSUM -> SBUF (+EPS, downcast), alternating DVE / ACT
            if h == 0:
                nc.vector.tensor_scalar_add(st2[:, :, h * TC:(h + 1) * TC],
                                            pt[:], EPS)
            else:
                nc.scalar.activation(st2[:, :, h * TC:(h + 1) * TC], pt[:],
                                     AF.Identity, bias=epsb[:, 0:1])
        eng = nc.sync if g % 2 == 0 else nc.scalar
        eng.dma_start(
            scratch[g * (C * SG):(g + 1) * (C * SG), :]
            .rearrange("(c bl) t -> c bl t", bl=SG),
            st2[:])

    # ---- Phase B/C interleaved with phase D (wavefront over gather batches)
    # blank rows (b, BLANK) are a fixed strided pattern -> plain DMA
    prb = praw.tile([PB, 8, T], BF16, tag="prb")
    nc.sync.dma_start(
        prb[:, 0, :],
        scratch[:].rearrange("(g cb) t -> g cb t", cb=C * SG)
        [:, BLANK * SG:BLANK * SG + SG, :])
    # batch q gathers label slots 8q..8q+7 -> praw[:, i, :]
    # (indirect DMA supports one offset per partition per call).  Batches
    # are issued lazily, interleaved with phase D on the Pool queue.
    prq: dict = {}

    def gather_batch(q):
        pr = praw.tile([PB, 8, T], BF16, tag="pr")
        for i in range(8):
            nc.gpsimd.indirect_dma_start(
                out=pr[:, i, :],
                out_offset=None,
                in_=scratch[:],
                in_offset=bass.IndirectOffsetOnAxis(
                    ap=bidx[:, 8 * q + i:8 * q + i + 1], axis=0))
        prq[q] = pr

    gather_batch(0)
    gather_batch(1)

    # gmax over blank + labels j0..j7 (from blank DMA + batch 0)
    nc.vector.tensor_reduce(
        gmax[:], prq[0][:].rearrange("p blk t -> p t blk"),
        axis=mybir.AxisListType.X, op=ALU.max)
    nc.vector.tensor_tensor(gmax[:], gmax[:], prb[:, 0, :], op=ALU.max)
    nc.vector.tensor_scalar_mul(gmax[:], gmax[:], float(np.exp(-RHAT)))
    ginv32 = cpool.tile([PB, T], F32, tag="ginv32")
    nc.vector.reciprocal(ginv32[:], gmax[:])
    nc.vector.tensor_copy(ginvb[:], ginv32[:])
    nc.sync.dma_start(ginv_out[:], ginvb[:])
    # blank slot scale
    nc.vector.tensor_mul(ptil[:, 0:T], prb[:, 0, :], ginvb[:])

    # ---- Phase D: 129-row s-sweep, 2 chunks, wavefront; scale-muls for
    # gather batch q are emitted just before the rows that need them.
    def sb(s):
        return (s + 2) * W

    def scale_batch(q):
        for i in range(8):
            slot = 1 + q * 8 + i
            nc.vector.tensor_mul(ptil[:, slot * T:(slot + 1) * T],
                                 prq[q][:, i, :], ginvb[:])

    for s in range(S):
        if s % 16 == 0 and s // 16 < 8:
            scale_batch(s // 16)
            if s // 16 + 2 < 8:
                gather_batch(s // 16 + 2)
        slot = 0 if s % 2 == 0 else 1 + (s - 1) // 2
        j = (s - 1) // 2
        t0 = 0 if s <= 1 else s // 2               # head prune
        t1 = T - (128 - s) // 2 if s < 128 else T  # tail prune
        n = t1 - t0
        if s % 2 == 1:
            # coupling c = m01_j * v^{s-2} + v^{s-1}: mask-mult on ACT
            # (dep on row s-2 -> off the serial chain), add on DVE
            c0 = cpool.tile([PB, T], BF16, tag="c")
            nc.scalar.activation(
                c0[:, :n],
                treg[:, sb(s - 2) + t0: sb(s - 2) + t1],
                AF.Identity, scale=m01[:, j:j + 1])
            nc.vector.tensor_add(
                c0[:, :n], c0[:, :n],
                treg[:, sb(s - 1) + t0: sb(s - 1) + t1])
            d0 = c0[:, :n]
        else:
            d0 = treg[:, sb(s - 1) + t0: sb(s - 1) + t1]
        nc.vector.tensor_tensor_scan(
            treg[:, sb(s) + 1 + t0: sb(s) + 1 + t1],
            d0, ptil[:, slot * T + t0: slot * T + t1],
            1.0 if s <= 1 else 0.0,
            op0=ALU.add, op1=ALU.mult)

    b127 = sb(127) + T
    b128 = sb(128) + T
    nc.vector.tensor_add(raw[:, 0:1], treg[:, b127:b127 + 1],
                         treg[:, b128:b128 + 1])
    nc.sync.dma_start(raw_out[:], raw[:])


_CACHE: dict = {}


def _build():
    nc = bacc.Bacc("TRN2", target_bir_lowering=False, debug=False,
                   num_devices=NCORES, num_swdge_queues=4)
    y_in = nc.dram_tensor("ypred", [PB, T, C], F32, kind="ExternalInput").ap()
    bidx_in = nc.dram_tensor("bidx", [PB, L], mybir.dt.int32,
                             kind="ExternalInput").ap()
    m01_in = nc.dram_tensor("m01", [PB, L], F32, kind="ExternalInput").ap()
    raw_out = nc.dram_tensor("raw", [PB, 1], F32, kind="ExternalOutput").ap()
    ginv_out = nc.dram_tensor("ginv", [PB, T], BF16, kind="ExternalOutput").ap()
    with tile.TileContext(nc) as tcx:
        with ExitStack() as ctx:
            _emit(ctx, tcx, y_in, bidx_in, m01_in, raw_out, ginv_out)
    nc.compile()
    return nc


def _run(in_maps, **kwargs):
    if "nc" not in _CACHE:
        _CACHE["nc"] = _build()
    return run_bass_kernel_spmd(_CACHE["nc"], in_maps,
                                core_ids=list(range(NCORES)), **kwargs)


def kernel(y_true: np.ndarray, y_pred: np.ndarray, **run_kwargs) -> np.ndarray:
    assert y_pred.shape == (B, T, C), y_pred.shape
    in_maps = []
    for c in range(NCORES):
        sl = slice(c * PB, (c + 1) * PB)
        prep = _host_prep(y_true[sl])
        in_maps.append({"ypred": np.ascontiguousarray(y_pred[sl], np.float32),
                        "bidx": prep["bidx"], "m01": prep["m01"]})
    res = _run(in_maps, **run_kwargs)
    raw = np.concatenate([res.results[c]["raw"] for c in range(NCORES)], axis=0)
    ginv = np.concatenate([res.results[c]["ginv"] for c in range(NCORES)],
                          axis=0).astype(np.float64)
    lng = np.log(ginv).sum(axis=1)
    loss = -(np.log(raw[:, 0].astype(np.float64)) - lng)
    if run_kwargs:
        kernel.last_results = res  # expose trace info to test harness
    return loss[:, None].astype(np.float32)


# revision 39
# speedup vs baseline: 1.0470x; 1.0158x over previous
"""CTC loss (keras ctc_batch_cost semantics) on 8 Trainium2 NeuronCores.

Data-parallel over batch: 1024 samples -> 8 cores x 128 samples
(one sample per SBUF partition).  Host prep is integer-only (gather
index tables + skip masks); all float work runs on device.

Device pipeline (per core):
  A. per 4-sample group x 2 T-halves: DMA load [t,(4b,c)] f32, DVE
     cast fp32->bf16 (+EPS), 4x PE transpose -> PSUM [c,t] f32, ACT
     copy PSUM->SBUF bf16; one contiguous 256KB store per group to an
     interleaved HBM scratch (row(b,c) = (b//4)*512 + c*4 + b%4, 512B
     full-T rows -> large DMA runs both directions).
  B. 9 SWDGE full-T gathers (elem 512B): rows (b, blank) and
     (b, lab_j) -> ptil[b, slot*T..] (slot 0 = blank, 1+j = label j).
  C. per-column scale from gather 0: gmax = max over slots 0..7,
     ginv = e^RHAT/gmax (fp32 recip, bf16); ptil = raw * ginv.
  D. s-sweep over the 129-row CTC lattice in 2 T-chunks, wavefront
     pipelined: row coupling (odd rows) as scalar_tensor_tensor on
     GpSimd, recursion v_t = (e_t + v_{t-1}) * p_t as
     tensor_tensor_scan on DVE (fp32 carry, bf16 treg).  Head/tail
     pruned to the reachable lattice region.  No mid-scan renorm:
     RHAT=0.28 centers the running product inside fp32 range.
Host assembles loss = -(log raw - sum log ginv) in f64.
"""
from contextlib import ExitStack

import numpy as np

import concourse.bass as bass
import concourse.tile as tile
from concourse import bacc, mybir, masks
from concourse.bass_utils import run_bass_kernel_spmd

F32 = mybir.dt.float32
F32R = mybir.dt.float32r
BF16 = mybir.dt.bfloat16
I16 = mybir.dt.int16
AF = mybir.ActivationFunctionType
ALU = mybir.AluOpType

B, T, C, L = 1024, 256, 128, 64
S = 2 * L + 1           # 129 extended states
NBLK = L + 1            # blank slot + 64 label slots
BLANK = C - 1
EPS = 1e-7
RHAT = 0.24             # per-step prob boost; keeps alpha in fp32 range
TC = 128                # T-chunk for the scan wavefront
W = T + 1               # treg slot width: col0 = v_{-1}, col 1+t = v_t
SLOTS = S + 2           # 2 permanent zero rows + 129 state rows
PB = 128                # samples per core
NCORES = 8
SG = 8                  # samples per load/transpose group
NG = PB // SG           # 16 groups


def _scratch_row(b, ch):
    return (b // SG) * (C * SG) + ch * SG + b % SG


def _host_prep(y_true_shard: np.ndarray):
    yt = y_true_shard.astype(np.int64)
    barange = np.arange(PB)
    # per-partition row indices for the indirect gather: bidx[b, j] = row of
    # (sample b, label j) in the interleaved scratch
    bidx = np.empty((PB, L), np.int32)
    for j in range(L):
        bidx[:, j] = _scratch_row(barange, yt[:, j])
    m01 = np.ones((PB, L), np.float32)
    m01[:, 1:] = (yt[:, 1:] != yt[:, :-1]).astype(np.float32)
    m01[:, 0] = 0.0
    return {"bidx": bidx, "m01": m01}


def _emit(ctx: ExitStack, tc: tile.TileContext, y_in, bidx_in, m01_in,
          raw_out, ginv_out):
    nc = tc.nc

    persist = ctx.enter_context(tc.tile_pool(name="persist", bufs=1))
    stage = ctx.enter_context(tc.tile_pool(name="stage", bufs=6))
    stg2 = ctx.enter_context(tc.tile_pool(name="stg2", bufs=4))
    praw = ctx.enter_context(tc.tile_pool(name="praw", bufs=4))
    cpool = ctx.enter_context(tc.tile_pool(name="cbuf", bufs=4))
    psum = ctx.enter_context(
        tc.tile_pool(name="psum", bufs=4, space=bass.MemorySpace.PSUM))
    dram = ctx.enter_context(tc.tile_pool(name="dram", bufs=1, space="DRAM"))

    bidx = persist.tile([PB, L], mybir.dt.int32)
    nc.sync.dma_start(bidx[:], bidx_in[:])
    m01 = persist.tile([PB, L], F32)
    nc.sync.dma_start(m01[:], m01_in[:])

    ident = persist.tile([128, 128], F32)
    masks.make_identity(nc, ident[:])
    epsb = persist.tile([PB, 1], F32)
    nc.vector.memset(epsb[:], EPS)

    treg = persist.tile([PB, SLOTS * W], BF16)
    # only the read-before-write region needs zeroing: the two permanent
    # zero rows, plus cols [0, 65) of every slot (covers col0 + the pruned
    # head triangle; max unwritten-but-read col is t0(s) <= 64)
    nc.vector.memset(treg[:, 0:2 * W], 0.0)
    nc.vector.memset(treg[:].rearrange("p (s w) -> p s w", w=W)[:, :, 0:65],
                     0.0)

    ptil = persist.tile([PB, NBLK * T], BF16)
    gmax = persist.tile([PB, T], F32)
    ginvb = persist.tile([PB, T], BF16)
    raw = persist.tile([PB, 1], F32)

    scratch = dram.tile([PB * C, T], BF16)

    # ---- Phase A: load -> PE transpose -> copy(+EPS, ->bf16) -> store
    for g in range(NG):
        b0 = g * SG
        st2 = stg2.tile([PB, SG, T], BF16, tag="st2")
        for h in range(2):
            ld = stage.tile([PB, SG * C], F32, tag="ld")
            eng = nc.sync if (2 * g + h) % 2 == 0 else nc.scalar
            eng.dma_start(
                ld[:].rearrange("p (b c) -> p b c", b=SG),
                y_in[b0:b0 + SG, h * TC:(h + 1) * TC, :]
                .rearrange("b t c -> t b c"))
            pt = psum.tile([PB, SG, TC], F32, tag="pt")
            for i in range(SG):
                nc.tensor.transpose(pt[:, i, :], ld[:, i * C:(i + 1) * C],
                                    ident[:])
            # PSUM -> SBUF (+EPS, downcast) on DVE: scalar only issues DMAs
            # in phase A, DVE is otherwise idle here
            nc.vector.tensor_scalar_add(st2[:, :, h * TC:(h + 1) * TC],
                                        pt[:], EPS)
        eng = nc.sync if g % 2 == 0 else nc.scalar
        eng.dma_start(
            scratch[g * (C * SG):(g + 1) * (C * SG), :]
            .rearrange("(c bl) t -> c bl t", bl=SG),
            st2[:])

    # ---- Phase B/C interleaved with phase D (wavefront over gather batches)
    # blank rows (b, BLANK) are a fixed strided pattern -> plain DMA
    prb = praw.tile([PB, 8, T], BF16, tag="prb")
    nc.sync.dma_start(
        prb[:, 0, :],
        scratch[:].rearrange("(g cb) t -> g cb t", cb=C * SG)
        [:, BLANK * SG:BLANK * SG + SG, :])
    # batch q gathers label slots 8q..8q+7 -> praw[:, i, :]
    # (indirect DMA supports one offset per partition per call).  Calls are
    # issued lazily, spread between phase-D rows on the Pool queue.
    prq: dict = {}

    def gather_call(q, i):
        if q not in prq:
            pr = praw.tile([PB, 8, T], BF16, tag="pr")
            prq[q] = pr
        nc.gpsimd.indirect_dma_start(
            out=prq[q][:, i, :],
            out_offset=None,
            in_=scratch[:],
            in_offset=bass.IndirectOffsetOnAxis(
                ap=bidx[:, 8 * q + i:8 * q + i + 1], axis=0))

    for q in (0, 1):
        for i in range(8):
            gather_call(q, i)

    # gmax over blank + labels j0..j7 (from blank DMA + batch 0)
    nc.vector.tensor_reduce(
        gmax[:], prq[0][:].rearrange("p blk t -> p t blk"),
        axis=mybir.AxisListType.X, op=ALU.max)
    nc.vector.tensor_tensor(gmax[:], gmax[:], prb[:, 0, :], op=ALU.max)
    nc.vector.tensor_scalar_mul(gmax[:], gmax[:], float(np.exp(-RHAT)))
    ginv32 = cpool.tile([PB, T], F32, tag="ginv32")
    nc.vector.reciprocal(ginv32[:], gmax[:])
    nc.vector.tensor_copy(ginvb[:], ginv32[:])
    nc.sync.dma_start(ginv_out[:], ginvb[:])
    # blank slot scale
    nc.vector.tensor_mul(ptil[:, 0:T], prb[:, 0, :], ginvb[:])

    # ---- Phase D: 129-row s-sweep, 2 chunks, wavefront; scale-muls for
    # gather batch q are emitted just before the rows that need them.
    def sb(s):
        return (s + 2) * W

    def scale_batch(q):
        for i in range(8):
            slot = 1 + q * 8 + i
            nc.vector.tensor_mul(ptil[:, slot * T:(slot + 1) * T],
                                 prq[q][:, i, :], ginvb[:])

    for s in range(S):
        if s % 16 == 0 and s // 16 < 8:
            scale_batch(s // 16)
        # spread the next-next batch's gather issues between rows
        if s % 2 == 0 and s // 16 + 2 < 8:
            gather_call(s // 16 + 2, (s % 16) // 2)
        slot = 0 if s % 2 == 0 else 1 + (s - 1) // 2
        j = (s - 1) // 2
        t0 = 0 if s <= 1 else s // 2               # head prune
        t1 = T - (128 - s) // 2 if s < 128 else T  # tail prune
        n = t1 - t0
        if s % 2 == 1:
            # coupling c = m01_j * v^{s-2} + v^{s-1}: mask-mult on ACT
            # (dep on row s-2 -> off the serial chain), add on DVE
            c0 = cpool.tile([PB, T], BF16, tag="c")
            nc.scalar.activation(
                c0[:, :n],
                treg[:, sb(s - 2) + t0: sb(s - 2) + t1],
                AF.Identity, scale=m01[:, j:j + 1])
            nc.vector.tensor_add(
                c0[:, :n], c0[:, :n],
                treg[:, sb(s - 1) + t0: sb(s - 1) + t1])
            d0 = c0[:, :n]
        else:
            d0 = treg[:, sb(s - 1) + t0: sb(s - 1) + t1]
        nc.vector.tensor_tensor_scan(
            treg[:, sb(s) + 1 + t0: sb(s) + 1 + t1],
            d0, ptil[:, slot * T + t0: slot * T + t1],
            1.0 if s <= 1 else 0.0,
            op0=ALU.add, op1=ALU.mult)

    b127 = sb(127) + T
    b128 = sb(128) + T
    nc.vector.tensor_add(raw[:, 0:1], treg[:, b127:b127 + 1],
                         treg[:, b128:b128 + 1])
    nc.sync.dma_start(raw_out[:], raw[:])


_CACHE: dict = {}


def _build():
    nc = bacc.Bacc("TRN2", target_bir_lowering=False, debug=False,
                   num_devices=NCORES, num_swdge_queues=4)
    y_in = nc.dram_tensor("ypred", [PB, T, C], F32, kind="ExternalInput").ap()
    bidx_in = nc.dram_tensor("bidx", [PB, L], mybir.dt.int32,
                             kind="ExternalInput").ap()
    m01_in = nc.dram_tensor("m01", [PB, L], F32, kind="ExternalInput").ap()
    raw_out = nc.dram_tensor("raw", [PB, 1], F32, kind="ExternalOutput").ap()
    ginv_out = nc.dram_tensor("ginv", [PB, T], BF16, kind="ExternalOutput").ap()
    with tile.TileContext(nc) as tcx:
        with ExitStack() as ctx:
            _emit(ctx, tcx, y_in, bidx_in, m01_in, raw_out, ginv_out)
    nc.compile()
    return nc


def _run(in_maps, **kwargs):
    if "nc" not in _CACHE:
        _CACHE["nc"] = _build()
    return run_bass_kernel_spmd(_CACHE["nc"], in_maps,
                                core_ids=list(range(NCORES)), **kwargs)


def kernel(y_true: np.ndarray, y_pred: np.ndarray, **run_kwargs) -> np.ndarray:
    assert y_pred.shape == (B, T, C), y_pred.shape
    in_maps = []
    for c in range(NCORES):
        sl = slice(c * PB, (c + 1) * PB)
        prep = _host_prep(y_true[sl])
        in_maps.append({"ypred": np.ascontiguousarray(y_pred[sl], np.float32),
                        "bidx": prep["bidx"], "m01": prep["m01"]})
    res = _run(in_maps, **run_kwargs)
    raw = np.concatenate([res.results[c]["raw"] for c in range(NCORES)], axis=0)
    ginv = np.concatenate([res.results[c]["ginv"] for c in range(NCORES)],
                          axis=0).astype(np.float64)
    lng = np.log(ginv).sum(axis=1)
    loss = -(np.log(raw[:, 0].astype(np.float64)) - lng)
    if run_kwargs:
        kernel.last_results = res  # expose trace info to test harness
    return loss[:, None].astype(np.float32)


# revision 45
# speedup vs baseline: 1.1911x; 1.1377x over previous
"""CTC loss (keras ctc_batch_cost semantics) on 8 Trainium2 NeuronCores.

Data-parallel over batch: 1024 samples -> 8 cores x 128 samples
(one sample per SBUF partition).  Host prep is integer-only (gather
index tables + skip masks); all float work runs on device.

Device pipeline (per core):
  A. per 4-sample group x 2 T-halves: DMA load [t,(4b,c)] f32, DVE
     cast fp32->bf16 (+EPS), 4x PE transpose -> PSUM [c,t] f32, ACT
     copy PSUM->SBUF bf16; one contiguous 256KB store per group to an
     interleaved HBM scratch (row(b,c) = (b//4)*512 + c*4 + b%4, 512B
     full-T rows -> large DMA runs both directions).
  B. 9 SWDGE full-T gathers (elem 512B): rows (b, blank) and
     (b, lab_j) -> ptil[b, slot*T..] (slot 0 = blank, 1+j = label j).
  C. per-column scale from gather 0: gmax = max over slots 0..7,
     ginv = e^RHAT/gmax (fp32 recip, bf16); ptil = raw * ginv.
  D. s-sweep over the 129-row CTC lattice in 2 T-chunks, wavefront
     pipelined: row coupling (odd rows) as scalar_tensor_tensor on
     GpSimd, recursion v_t = (e_t + v_{t-1}) * p_t as
     tensor_tensor_scan on DVE (fp32 carry, bf16 treg).  Head/tail
     pruned to the reachable lattice region.  No mid-scan renorm:
     RHAT=0.28 centers the running product inside fp32 range.
Host assembles loss = -(log raw - sum log ginv) in f64.
"""
from contextlib import ExitStack

import numpy as np

import concourse.bass as bass
import concourse.tile as tile
from concourse import bacc, mybir, masks
from concourse.bass_utils import run_bass_kernel_spmd

F32 = mybir.dt.float32
F32R = mybir.dt.float32r
BF16 = mybir.dt.bfloat16
I16 = mybir.dt.int16
AF = mybir.ActivationFunctionType
ALU = mybir.AluOpType

B, T, C, L = 1024, 256, 128, 64
S = 2 * L + 1           # 129 extended states
NBLK = L + 1            # blank slot + 64 label slots
BLANK = C - 1
EPS = 1e-7
RHAT = 0.24             # per-step prob boost; keeps alpha in fp32 range
TC = 128                # T-chunk for the scan wavefront
W = T + 1               # treg slot width: col0 = v_{-1}, col 1+t = v_t
SLOTS = S + 2           # 2 permanent zero rows + 129 state rows
PB = 128                # samples per core
NCORES = 8
SG = 8                  # samples per load/transpose group
NG = PB // SG           # 16 groups


def _scratch_row(b, ch):
    return (b // SG) * (C * SG) + ch * SG + b % SG


def _host_prep(y_true_shard: np.ndarray):
    yt = y_true_shard.astype(np.int64)
    barange = np.arange(PB)
    # SWDGE gather table: slot-major (j, b) -> scratch row of (b, label j),
    # wrapped into 16 partitions and replicated across the 8 DSP cores
    idx_flat = np.empty(L * PB, np.int32)
    for j in range(L):
        idx_flat[j * PB:(j + 1) * PB] = _scratch_row(barange, yt[:, j])
    table16 = idx_flat.reshape(L * PB // 16, 16).T          # [16, 512]
    idxs = np.tile(table16, (8, 1)).astype(np.int16)        # [128, 512]
    m01 = np.ones((PB, L), np.float32)
    m01[:, 1:] = (yt[:, 1:] != yt[:, :-1]).astype(np.float32)
    m01[:, 0] = 0.0
    return {"idxs": idxs, "m01": m01}


def _emit(ctx: ExitStack, tc: tile.TileContext, y_in, idxs_in, m01_in,
          raw_out, ginv_out):
    nc = tc.nc

    persist = ctx.enter_context(tc.tile_pool(name="persist", bufs=1))
    stage = ctx.enter_context(tc.tile_pool(name="stage", bufs=6))
    stg2 = ctx.enter_context(tc.tile_pool(name="stg2", bufs=4))
    praw = ctx.enter_context(tc.tile_pool(name="praw", bufs=4))
    cpool = ctx.enter_context(tc.tile_pool(name="cbuf", bufs=4))
    psum = ctx.enter_context(
        tc.tile_pool(name="psum", bufs=4, space=bass.MemorySpace.PSUM))
    dram = ctx.enter_context(tc.tile_pool(name="dram", bufs=1, space="DRAM"))

    idxs = persist.tile([PB, L * PB // 16], I16)
    nc.sync.dma_start(idxs[:], idxs_in[:])
    m01 = persist.tile([PB, L], F32)
    nc.sync.dma_start(m01[:], m01_in[:])

    ident = persist.tile([128, 128], F32)
    masks.make_identity(nc, ident[:])
    epsb = persist.tile([PB, 1], F32)
    nc.vector.memset(epsb[:], EPS)

    treg = persist.tile([PB, SLOTS * W], BF16)
    # only the read-before-write region needs zeroing: the two permanent
    # zero rows, plus cols [0, 65) of every slot (covers col0 + the pruned
    # head triangle; max unwritten-but-read col is t0(s) <= 64)
    nc.vector.memset(treg[:, 0:2 * W], 0.0)
    nc.vector.memset(treg[:].rearrange("p (s w) -> p s w", w=W)[:, :, 0:65],
                     0.0)

    ptil = persist.tile([PB, NBLK * T], BF16)
    gmax = persist.tile([PB, T], F32)
    ginvb = persist.tile([PB, T], BF16)
    raw = persist.tile([PB, 1], F32)

    scratch = dram.tile([PB * C, T], BF16)

    # ---- Phase A: load -> PE transpose -> copy(+EPS, ->bf16) -> store
    for g in range(NG):
        b0 = g * SG
        st2 = stg2.tile([PB, SG, T], BF16, tag="st2")
        for h in range(2):
            ld = stage.tile([PB, SG * C], F32, tag="ld")
            eng = nc.sync if (2 * g + h) % 2 == 0 else nc.scalar
            eng.dma_start(
                ld[:].rearrange("p (b c) -> p b c", b=SG),
                y_in[b0:b0 + SG, h * TC:(h + 1) * TC, :]
                .rearrange("b t c -> t b c"))
            pt = psum.tile([PB, SG, TC], F32, tag="pt")
            for i in range(SG):
                nc.tensor.transpose(pt[:, i, :], ld[:, i * C:(i + 1) * C],
                                    ident[:])
            # PSUM -> SBUF (+EPS, downcast) on DVE: scalar only issues DMAs
            # in phase A, DVE is otherwise idle here
            nc.vector.tensor_scalar_add(st2[:, :, h * TC:(h + 1) * TC],
                                        pt[:], EPS)
        eng = nc.sync if g % 2 == 0 else nc.scalar
        eng.dma_start(
            scratch[g * (C * SG):(g + 1) * (C * SG), :]
            .rearrange("(c bl) t -> c bl t", bl=SG),
            st2[:])

    # ---- Phase B/C interleaved with phase D (wavefront over gather batches)
    # blank rows (b, BLANK) are a fixed strided pattern -> plain DMA
    prb = praw.tile([PB, 8, T], BF16, tag="prb")
    nc.sync.dma_start(
        prb[:, 0, :],
        scratch[:].rearrange("(g cb) t -> g cb t", cb=C * SG)
        [:, BLANK * SG:BLANK * SG + SG, :])
    # batch q gathers label slots 1+8q..8+8q via one SWDGE call (1024 idxs,
    # full-T 512B elems), rotating the 4 SWDGE queues
    prq: dict = {}

    def gather_batch(q):
        pr = praw.tile([PB, 8, T], BF16, tag="pr")
        prq[q] = pr
        nc.gpsimd.dma_gather(
            pr[:], scratch[:], idxs[:, 64 * q:64 * q + 64],
            num_idxs=8 * PB, num_idxs_reg=8 * PB,
            elem_size=T, queue_num=q % 4)

    gather_batch(0)
    gather_batch(1)
    gather_batch(2)

    # gmax over blank + labels j0..j7 (from blank DMA + batch 0)
    nc.vector.tensor_reduce(
        gmax[:], prq[0][:].rearrange("p blk t -> p t blk"),
        axis=mybir.AxisListType.X, op=ALU.max)
    nc.vector.tensor_tensor(gmax[:], gmax[:], prb[:, 0, :], op=ALU.max)
    nc.vector.tensor_scalar_mul(gmax[:], gmax[:], float(np.exp(-RHAT)))
    ginv32 = cpool.tile([PB, T], F32, tag="ginv32")
    nc.vector.reciprocal(ginv32[:], gmax[:])
    nc.vector.tensor_copy(ginvb[:], ginv32[:])
    nc.sync.dma_start(ginv_out[:], ginvb[:])
    # blank slot scale
    nc.vector.tensor_mul(ptil[:, 0:T], prb[:, 0, :], ginvb[:])

    # ---- Phase D: 129-row s-sweep, 2 chunks, wavefront; scale-muls for
    # gather batch q are emitted just before the rows that need them.
    def sb(s):
        return (s + 2) * W

    def scale_batch(q):
        for i in range(8):
            slot = 1 + q * 8 + i
            nc.vector.tensor_mul(ptil[:, slot * T:(slot + 1) * T],
                                 prq[q][:, i, :], ginvb[:])

    for s in range(S):
        if s % 16 == 0 and s // 16 < 8:
            scale_batch(s // 16)
            if s // 16 + 3 < 8:
                gather_batch(s // 16 + 3)
        slot = 0 if s % 2 == 0 else 1 + (s - 1) // 2
        j = (s - 1) // 2
        t0 = 0 if s <= 1 else s // 2               # head prune
        t1 = T - (128 - s) // 2 if s < 128 else T  # tail prune
        n = t1 - t0
        if s % 2 == 1:
            # coupling c = m01_j * v^{s-2} + v^{s-1}: mask-mult on ACT
            # (dep on row s-2 -> off the serial chain), add on DVE
            c0 = cpool.tile([PB, T], BF16, tag="c")
            nc.scalar.activation(
                c0[:, :n],
                treg[:, sb(s - 2) + t0: sb(s - 2) + t1],
                AF.Identity, scale=m01[:, j:j + 1])
            nc.vector.tensor_add(
                c0[:, :n], c0[:, :n],
                treg[:, sb(s - 1) + t0: sb(s - 1) + t1])
            d0 = c0[:, :n]
        else:
            d0 = treg[:, sb(s - 1) + t0: sb(s - 1) + t1]
        nc.vector.tensor_tensor_scan(
            treg[:, sb(s) + 1 + t0: sb(s) + 1 + t1],
            d0, ptil[:, slot * T + t0: slot * T + t1],
            1.0 if s <= 1 else 0.0,
            op0=ALU.add, op1=ALU.mult)

    b127 = sb(127) + T
    b128 = sb(128) + T
    nc.vector.tensor_add(raw[:, 0:1], treg[:, b127:b127 + 1],
                         treg[:, b128:b128 + 1])
    nc.sync.dma_start(raw_out[:], raw[:])


_CACHE: dict = {}


def _build():
    nc = bacc.Bacc("TRN2", target_bir_lowering=False, debug=False,
                   num_devices=NCORES, num_swdge_queues=4)
    y_in = nc.dram_tensor("ypred", [PB, T, C], F32, kind="ExternalInput").ap()
    idxs_in = nc.dram_tensor("idxs", [PB, L * PB // 16], I16,
                             kind="ExternalInput").ap()
    m01_in = nc.dram_tensor("m01", [PB, L], F32, kind="ExternalInput").ap()
    raw_out = nc.dram_tensor("raw", [PB, 1], F32, kind="ExternalOutput").ap()
    ginv_out = nc.dram_tensor("ginv", [PB, T], BF16, kind="ExternalOutput").ap()
    with tile.TileContext(nc) as tcx:
        with ExitStack() as ctx:
            _emit(ctx, tcx, y_in, idxs_in, m01_in, raw_out, ginv_out)
    nc.compile()
    return nc


def _run(in_maps, **kwargs):
    if "nc" not in _CACHE:
        _CACHE["nc"] = _build()
    return run_bass_kernel_spmd(_CACHE["nc"], in_maps,
                                core_ids=list(range(NCORES)), **kwargs)


def kernel(y_true: np.ndarray, y_pred: np.ndarray, **run_kwargs) -> np.ndarray:
    assert y_pred.shape == (B, T, C), y_pred.shape
    in_maps = []
    for c in range(NCORES):
        sl = slice(c * PB, (c + 1) * PB)
        prep = _host_prep(y_true[sl])
        in_maps.append({"ypred": np.ascontiguousarray(y_pred[sl], np.float32),
                        "idxs": prep["idxs"], "m01": prep["m01"]})
    res = _run(in_maps, **run_kwargs)
    raw = np.concatenate([res.results[c]["raw"] for c in range(NCORES)], axis=0)
    ginv = np.concatenate([res.results[c]["ginv"] for c in range(NCORES)],
                          axis=0).astype(np.float64)
    lng = np.log(ginv).sum(axis=1)
    loss = -(np.log(raw[:, 0].astype(np.float64)) - lng)
    if run_kwargs:
        kernel.last_results = res  # expose trace info to test harness
    return loss[:, None].astype(np.float32)


# revision 48
# speedup vs baseline: 1.1923x; 1.0010x over previous
"""CTC loss (keras ctc_batch_cost semantics) on 8 Trainium2 NeuronCores.

Data-parallel over batch: 1024 samples -> 8 cores x 128 samples
(one sample per SBUF partition).  Host prep is integer-only (gather
index tables + skip masks); all float work runs on device.

Device pipeline (per core):
  A. per 4-sample group x 2 T-halves: DMA load [t,(4b,c)] f32, DVE
     cast fp32->bf16 (+EPS), 4x PE transpose -> PSUM [c,t] f32, ACT
     copy PSUM->SBUF bf16; one contiguous 256KB store per group to an
     interleaved HBM scratch (row(b,c) = (b//4)*512 + c*4 + b%4, 512B
     full-T rows -> large DMA runs both directions).
  B. 9 SWDGE full-T gathers (elem 512B): rows (b, blank) and
     (b, lab_j) -> ptil[b, slot*T..] (slot 0 = blank, 1+j = label j).
  C. per-column scale from gather 0: gmax = max over slots 0..7,
     ginv = e^RHAT/gmax (fp32 recip, bf16); ptil = raw * ginv.
  D. s-sweep over the 129-row CTC lattice in 2 T-chunks, wavefront
     pipelined: row coupling (odd rows) as scalar_tensor_tensor on
     GpSimd, recursion v_t = (e_t + v_{t-1}) * p_t as
     tensor_tensor_scan on DVE (fp32 carry, bf16 treg).  Head/tail
     pruned to the reachable lattice region.  No mid-scan renorm:
     RHAT=0.28 centers the running product inside fp32 range.
Host assembles loss = -(log raw - sum log ginv) in f64.
"""
from contextlib import ExitStack

import numpy as np

import concourse.bass as bass
import concourse.tile as tile
from concourse import bacc, mybir, masks
from concourse.bass_utils import run_bass_kernel_spmd

F32 = mybir.dt.float32
F32R = mybir.dt.float32r
BF16 = mybir.dt.bfloat16
I16 = mybir.dt.int16
AF = mybir.ActivationFunctionType
ALU = mybir.AluOpType

B, T, C, L = 1024, 256, 128, 64
S = 2 * L + 1           # 129 extended states
NBLK = L + 1            # blank slot + 64 label slots
BLANK = C - 1
EPS = 1e-7
RHAT = 0.20             # per-step prob boost; keeps alpha in fp32 range
TC = 128                # T-chunk for the scan wavefront
W = T + 1               # treg slot width: col0 = v_{-1}, col 1+t = v_t
SLOTS = S + 2           # 2 permanent zero rows + 129 state rows
PB = 128                # samples per core
NCORES = 8
SG = 8                  # samples per load/transpose group
NG = PB // SG           # 16 groups


def _scratch_row(b, ch):
    return (b // SG) * (C * SG) + ch * SG + b % SG


def _host_prep(y_true_shard: np.ndarray):
    yt = y_true_shard.astype(np.int64)
    barange = np.arange(PB)
    # SWDGE gather table: slot-major (j, b) -> scratch row of (b, label j),
    # wrapped into 16 partitions and replicated across the 8 DSP cores
    idx_flat = np.empty(L * PB, np.int32)
    for j in range(L):
        idx_flat[j * PB:(j + 1) * PB] = _scratch_row(barange, yt[:, j])
    table16 = idx_flat.reshape(L * PB // 16, 16).T          # [16, 512]
    idxs = np.tile(table16, (8, 1)).astype(np.int16)        # [128, 512]
    m01 = np.ones((PB, L), np.float32)
    m01[:, 1:] = (yt[:, 1:] != yt[:, :-1]).astype(np.float32)
    m01[:, 0] = 0.0
    return {"idxs": idxs, "m01": m01}


def _emit(ctx: ExitStack, tc: tile.TileContext, y_in, idxs_in, m01_in,
          raw_out, ginv_out):
    nc = tc.nc

    persist = ctx.enter_context(tc.tile_pool(name="persist", bufs=1))
    stage = ctx.enter_context(tc.tile_pool(name="stage", bufs=6))
    stg2 = ctx.enter_context(tc.tile_pool(name="stg2", bufs=4))
    praw = ctx.enter_context(tc.tile_pool(name="praw", bufs=4))
    cpool = ctx.enter_context(tc.tile_pool(name="cbuf", bufs=4))
    psum = ctx.enter_context(
        tc.tile_pool(name="psum", bufs=4, space=bass.MemorySpace.PSUM))
    dram = ctx.enter_context(tc.tile_pool(name="dram", bufs=1, space="DRAM"))

    idxs = persist.tile([PB, L * PB // 16], I16)
    nc.sync.dma_start(idxs[:], idxs_in[:])
    m01 = persist.tile([PB, L], F32)
    nc.sync.dma_start(m01[:], m01_in[:])

    ident = persist.tile([128, 128], F32)
    masks.make_identity(nc, ident[:])
    epsb = persist.tile([PB, 1], F32)
    nc.vector.memset(epsb[:], EPS)

    treg = persist.tile([PB, SLOTS * W], BF16)
    # only the read-before-write region needs zeroing: the two permanent
    # zero rows, plus cols [0, 65) of every slot (covers col0 + the pruned
    # head triangle; max unwritten-but-read col is t0(s) <= 64)
    nc.vector.memset(treg[:, 0:2 * W], 0.0)
    nc.vector.memset(treg[:].rearrange("p (s w) -> p s w", w=W)[:, :, 0:65],
                     0.0)

    ptil = persist.tile([PB, NBLK * T], BF16)
    gmax = persist.tile([PB, T], F32)
    ginvb = persist.tile([PB, T], BF16)
    raw = persist.tile([PB, 1], F32)

    scratch = dram.tile([PB * C, T], BF16)

    # ---- Phase A: load -> PE transpose -> copy(+EPS, ->bf16) -> store
    for g in range(NG):
        b0 = g * SG
        st2 = stg2.tile([PB, SG, T], BF16, tag="st2")
        for h in range(2):
            ld = stage.tile([PB, SG * C], F32, tag="ld")
            # split each load across both HWDGE queues to halve arrival
            # latency for the PE transposes
            hb = SG // 2
            nc.sync.dma_start(
                ld[:, :hb * C].rearrange("p (b c) -> p b c", b=hb),
                y_in[b0:b0 + hb, h * TC:(h + 1) * TC, :]
                .rearrange("b t c -> t b c"))
            nc.scalar.dma_start(
                ld[:, hb * C:].rearrange("p (b c) -> p b c", b=hb),
                y_in[b0 + hb:b0 + SG, h * TC:(h + 1) * TC, :]
                .rearrange("b t c -> t b c"))
            pt = psum.tile([PB, SG, TC], F32, tag="pt")
            for i in range(SG):
                nc.tensor.transpose(pt[:, i, :], ld[:, i * C:(i + 1) * C],
                                    ident[:])
            # PSUM -> SBUF (+EPS, downcast) on DVE: scalar only issues DMAs
            # in phase A, DVE is otherwise idle here
            nc.vector.tensor_scalar_add(st2[:, :, h * TC:(h + 1) * TC],
                                        pt[:], EPS)
        eng = nc.sync if g % 2 == 0 else nc.scalar
        eng.dma_start(
            scratch[g * (C * SG):(g + 1) * (C * SG), :]
            .rearrange("(c bl) t -> c bl t", bl=SG),
            st2[:])

    # ---- Phase B/C interleaved with phase D (wavefront over gather batches)
    # blank rows (b, BLANK) are a fixed strided pattern -> plain DMA
    prb = praw.tile([PB, 8, T], BF16, tag="prb")
    nc.sync.dma_start(
        prb[:, 0, :],
        scratch[:].rearrange("(g cb) t -> g cb t", cb=C * SG)
        [:, BLANK * SG:BLANK * SG + SG, :])
    # batch q gathers label slots 1+8q..8+8q via one SWDGE call (1024 idxs,
    # full-T 512B elems), rotating the 4 SWDGE queues
    prq: dict = {}

    def gather_batch(q):
        pr = praw.tile([PB, 8, T], BF16, tag="pr")
        prq[q] = pr
        nc.gpsimd.dma_gather(
            pr[:], scratch[:], idxs[:, 64 * q:64 * q + 64],
            num_idxs=8 * PB, num_idxs_reg=8 * PB,
            elem_size=T, queue_num=q % 4)

    gather_batch(0)
    gather_batch(1)
    gather_batch(2)

    # gmax over labels j0..j7 (batch 0 only -> blank DMA off the lead)
    nc.vector.tensor_reduce(
        gmax[:], prq[0][:].rearrange("p blk t -> p t blk"),
        axis=mybir.AxisListType.X, op=ALU.max)
    nc.vector.tensor_scalar_mul(gmax[:], gmax[:], float(np.exp(-RHAT)))
    ginv32 = cpool.tile([PB, T], F32, tag="ginv32")
    nc.vector.reciprocal(ginv32[:], gmax[:])
    nc.vector.tensor_copy(ginvb[:], ginv32[:])
    nc.sync.dma_start(ginv_out[:], ginvb[:])
    # blank slot scale
    nc.vector.tensor_mul(ptil[:, 0:T], prb[:, 0, :], ginvb[:])

    # ---- Phase D: 129-row s-sweep, 2 chunks, wavefront; scale-muls for
    # gather batch q are emitted just before the rows that need them.
    def sb(s):
        return (s + 2) * W

    def scale_batch(q):
        for i in range(8):
            slot = 1 + q * 8 + i
            nc.vector.tensor_mul(ptil[:, slot * T:(slot + 1) * T],
                                 prq[q][:, i, :], ginvb[:])

    for s in range(S):
        if s % 16 == 0 and s // 16 < 8:
            scale_batch(s // 16)
            if s // 16 + 3 < 8:
                gather_batch(s // 16 + 3)
        slot = 0 if s % 2 == 0 else 1 + (s - 1) // 2
        j = (s - 1) // 2
        t0 = 0 if s <= 1 else s // 2               # head prune
        t1 = T - (128 - s) // 2 if s < 128 else T  # tail prune
        n = t1 - t0
        if s % 2 == 1:
            # coupling c = m01_j * v^{s-2} + v^{s-1}: mask-mult on ACT
            # (dep on row s-2 -> off the serial chain), add on DVE
            c0 = cpool.tile([PB, T], BF16, tag="c")
            nc.scalar.activation(
                c0[:, :n],
                treg[:, sb(s - 2) + t0: sb(s - 2) + t1],
                AF.Identity, scale=m01[:, j:j + 1])
            nc.vector.tensor_add(
                c0[:, :n], c0[:, :n],
                treg[:, sb(s - 1) + t0: sb(s - 1) + t1])
            d0 = c0[:, :n]
        else:
            d0 = treg[:, sb(s - 1) + t0: sb(s - 1) + t1]
        nc.vector.tensor_tensor_scan(
            treg[:, sb(s) + 1 + t0: sb(s) + 1 + t1],
            d0, ptil[:, slot * T + t0: slot * T + t1],
            1.0 if s <= 1 else 0.0,
            op0=ALU.add, op1=ALU.mult)

    b127 = sb(127) + T
    b128 = sb(128) + T
    nc.vector.tensor_add(raw[:, 0:1], treg[:, b127:b127 + 1],
                         treg[:, b128:b128 + 1])
    nc.sync.dma_start(raw_out[:], raw[:])


_CACHE: dict = {}


def _build():
    nc = bacc.Bacc("TRN2", target_bir_lowering=False, debug=False,
                   num_devices=NCORES, num_swdge_queues=4)
    y_in = nc.dram_tensor("ypred", [PB, T, C], F32, kind="ExternalInput").ap()
    idxs_in = nc.dram_tensor("idxs", [PB, L * PB // 16], I16,
                             kind="ExternalInput").ap()
    m01_in = nc.dram_tensor("m01", [PB, L], F32, kind="ExternalInput").ap()
    raw_out = nc.dram_tensor("raw", [PB, 1], F32, kind="ExternalOutput").ap()
    ginv_out = nc.dram_tensor("ginv", [PB, T], BF16, kind="ExternalOutput").ap()
    with tile.TileContext(nc) as tcx:
        with ExitStack() as ctx:
            _emit(ctx, tcx, y_in, idxs_in, m01_in, raw_out, ginv_out)
    nc.compile()
    return nc


def _run(in_maps, **kwargs):
    if "nc" not in _CACHE:
        _CACHE["nc"] = _build()
    return run_bass_kernel_spmd(_CACHE["nc"], in_maps,
                                core_ids=list(range(NCORES)), **kwargs)


def kernel(y_true: np.ndarray, y_pred: np.ndarray, **run_kwargs) -> np.ndarray:
    assert y_pred.shape == (B, T, C), y_pred.shape
    in_maps = []
    for c in range(NCORES):
        sl = slice(c * PB, (c + 1) * PB)
        prep = _host_prep(y_true[sl])
        in_maps.append({"ypred": np.ascontiguousarray(y_pred[sl], np.float32),
                        "idxs": prep["idxs"], "m01": prep["m01"]})
    res = _run(in_maps, **run_kwargs)
    raw = np.concatenate([res.results[c]["raw"] for c in range(NCORES)], axis=0)
    ginv = np.concatenate([res.results[c]["ginv"] for c in range(NCORES)],
                          axis=0).astype(np.float64)
    lng = np.log(ginv).sum(axis=1)
    loss = -(np.log(raw[:, 0].astype(np.float64)) - lng)
    if run_kwargs:
        kernel.last_results = res  # expose trace info to test harness
    return loss[:, None].astype(np.float32)
